# revision 8
# baseline (speedup 1.0000x reference)
"""Distributed Trainium2 kernel for nn_ContrastiveLoss (survival contrastive loss).

Strategy (8 NeuronCores, symmetric block-pair decomposition):
  host: quantile-bin rows into 4 risk groups, stable-sort rows by group,
        L2-normalize, scale x64 and quantize to fp8-e4m3; build z^T in
        DoubleRow layout [128, 2, N]; each core gets a column-rotated copy
        (rotation by c*512) so one static SPMD program covers all 136
        unordered 512x512 block-pairs of the symmetric sim matrix
        (template pairs: i<=j, (i+j) mod 16 in {0,1} -> 17 pairs/core).
  device (per core): per block-pair chunk [128 rows x 512 cols]:
        2 fp8 DoubleRow matmuls (K=256 each) -> psum; ACT exp(10*s-10)
        over both halves of a super-block in one [128,2,512] instruction
        -> bf16 SBUF; DVE row-sum reduce (off-diag pairs); ones-matmul
        col-sums accumulated in psum (= row sums of the transposed block,
        by symmetry); fused mask+reduce diag extraction for the 2 diagonal
        template blocks; gpsimd stages colsum psum rows to SBUF.
  host: scatter row/col partial sums into den/pos by risk group (f64),
        subtract extracted diagonal, loss = mean(ln den - ln pos).
"""
import sys

sys.path.insert(0, "/opt/trn_rl_repo")
import numpy as np

N, D, G, NCORES = 8192, 512, 4, 8
NB = 16          # number of 512-row/col blocks
BS = 512         # block size
TEMP = 0.1
ZSCALE = 64.0    # fp8 pre-scale (avoids subnormals)

# super-blocks: groups of (1 or 2) template pairs sharing one psum tile.
# template = {(i,j): i<=j, (i+j) mod 16 in {0,1}}; diag pairs first.
SBS = [
    [(0, 0), (0, 1)],
    [(8, 8), (8, 9)],
    [(1, 15), (2, 15)],
    [(2, 14), (3, 14)],
    [(3, 13), (4, 13)],
    [(4, 12), (5, 12)],
    [(5, 11), (6, 11)],
    [(6, 10), (7, 10)],
    [(7, 9)],
]
# column-block DMA order: prefix-feasible for the SB sequence above
LOAD_ORDER = [0, 1, 8, 9, 2, 15, 3, 14, 4, 13, 5, 12, 6, 11, 7, 10]

_built = None


def _build():
    from concourse import bacc, tile, mybir

    nc = bacc.Bacc(None, target_bir_lowering=False)
    f32 = mybir.dt.float32
    bf16 = mybir.dt.bfloat16
    fp8 = mybir.dt.float8e4
    u8 = mybir.dt.uint8
    AF = mybir.ActivationFunctionType
    AX = mybir.AxisListType
    ADD = mybir.AluOpType.add
    MUL = mybir.AluOpType.mult
    DR = mybir.MatmulPerfMode.DoubleRow

    zA = nc.dram_tensor("zA", [128, 2, N], u8, kind="ExternalInput")
    zB = nc.dram_tensor("zB", [128, 2, N], u8, kind="ExternalInput")
    idm = nc.dram_tensor("idm", [128, 128], f32, kind="ExternalInput")
    rso = nc.dram_tensor("rso", [128, 80], f32, kind="ExternalOutput")
    cso = nc.dram_tensor("cso", [1, 8704], f32, kind="ExternalOutput")

    ACT_SCALE = (1.0 / TEMP) / (ZSCALE * ZSCALE)

    with tile.TileContext(nc) as tc:
        with tc.tile_pool(name="zt", bufs=1) as ztp, \
             tc.tile_pool(name="cst", bufs=1) as cstp, \
             tc.tile_pool(name="eb", bufs=3) as ebp, \
             tc.tile_pool(name="sc", bufs=2) as scp, \
             tc.tile_pool(name="pm", bufs=2, space="PSUM") as pmp, \
             tc.tile_pool(name="pc", bufs=2, space="PSUM") as pcp:

            # preload exp/ln act table once (set 6)
            nc.scalar.add_instruction(
                mybir.InstLoadActFuncSet(
                    name=nc.get_next_instruction_name(),
                    act_func_set_id=6, ins=[], outs=[]))

            zAs = ztp.tile([128, 2, N], fp8, name="zAs")
            zBs = ztp.tile([128, 2, N], fp8, name="zBs")
            for cb in LOAD_ORDER:
                cs = slice(cb * BS, (cb + 1) * BS)
                nc.sync.dma_start(zAs[:, :, cs], zA[:, :, cs].bitcast(fp8))
                nc.sync.dma_start(zBs[:, :, cs], zB[:, :, cs].bitcast(fp8))

            idmt = cstp.tile([128, 128], f32, name="idmt")
            nc.sync.dma_start(idmt[:], idm[:])
            onest = cstp.tile([128, 128], bf16, name="onest")
            nc.vector.memset(onest[:], 1.0)
            biasm = cstp.tile([128, 1], f32, name="biasm")
            nc.vector.memset(biasm[:], -10.0)
            rst = cstp.tile([128, 80], f32, name="rst")
            nc.vector.memset(rst[:], 0.0)
            csb = cstp.tile([1, 8704], f32, name="csb")

            zts = (zAs, zBs)

            def emit_sim(k, sb, r, pm):
                for h, (bi, bj) in enumerate(sb):
                    ls = slice(bi * BS + r * 128, bi * BS + (r + 1) * 128)
                    rs_ = slice(bj * BS, (bj + 1) * BS)
                    for t in range(2):
                        nc.tensor.matmul(pm[:, h, :], zts[t][:, :, ls],
                                         zts[t][:, :, rs_],
                                         start=(t == 0), stop=(t == 1),
                                         perf_mode=DR)

            # chunks whose row sums ride the ACT accumulator (DVE relief):
            # two single-half exp instructions instead of one double
            ACT_ACCUM_CHUNKS = {(2, 1), (4, 1), (6, 1)}

            def emit_post(k, sb, r, pm, cst):
                nh = len(sb)
                eb = ebp.tile([128, 2, 512], bf16, tag="eb", name=f"eb{k}_{r}")
                if (k, r) in ACT_ACCUM_CHUNKS and nh == 2 and k not in (0, 1):
                    for h in range(2):
                        nc.scalar.activation(
                            eb[:, h, :], pm[:, h, :], AF.Exp,
                            bias=biasm[:], scale=ACT_SCALE,
                            accum_out=rst[:, k * 8 + r * 2 + h:
                                          k * 8 + r * 2 + h + 1])
                    for h in range(nh):
                        nc.tensor.matmul(cst[:, h, :], onest[:], eb[:, h, :],
                                         start=(r == 0), stop=(r == 3))
                    return
                nc.scalar.activation(eb[:, :nh, :], pm[:, :nh, :], AF.Exp,
                                     bias=biasm[:], scale=ACT_SCALE)
                if k in (0, 1):
                    # diag pair in half 0: its row sums = col sums (symmetric
                    # block) -> host reads them from cso; reduce half 1 only.
                    nc.vector.tensor_reduce(
                        rst[:, k * 8 + r * 2 + 1: k * 8 + r * 2 + 2],
                        eb[:, 1:2, :], AX.X, ADD)
                    # diag extraction: mask-mul then row-reduce
                    sc = scp.tile([128, 128], bf16, tag="sc", name=f"sc{k}_{r}")
                    nc.vector.tensor_mul(sc[:], eb[:, 0, r * 128:(r + 1) * 128],
                                         idmt[:])
                    nc.vector.tensor_reduce(
                        rst[:, 72 + k * 4 + r: 73 + k * 4 + r],
                        sc[:], AX.X, ADD)
                else:
                    nc.vector.tensor_reduce(
                        rst[:, k * 8 + r * 2: k * 8 + r * 2 + nh],
                        eb[:, :nh, :], AX.X, ADD)
                for h in range(nh):
                    nc.tensor.matmul(cst[:, h, :], onest[:], eb[:, h, :],
                                     start=(r == 0), stop=(r == 3))
                if r == 3:
                    # stage colsum psum row to SBUF (gpsimd cannot read psum)
                    nc.vector.tensor_copy(csb[0:1, k * 1024: k * 1024 + nh * 512],
                                          cst[0:1, :nh, :])

            # software-pipelined emission: sim(r+1) goes ahead of post(r) so
            # the PE never sits behind an ACT it doesn't depend on
            pend = None
            for k, sb in enumerate(SBS):
                cst = pcp.tile([128, 2, 512], f32, tag="cst", name=f"cst{k}")
                for r in range(4):
                    pm = pmp.tile([128, 2, 512], f32, tag="pm",
                                  name=f"pm{k}_{r}")
                    emit_sim(k, sb, r, pm)
                    if pend is not None:
                        emit_post(*pend)
                    pend = (k, sb, r, pm, cst)
            emit_post(*pend)
            nc.sync.dma_start(cso[:], csb[:])
            nc.sync.dma_start(rso[:], rst[:])

    nc.finalize()
    return nc


def _get_built():
    global _built
    if _built is None:
        _built = _build()
    return _built


_E4M3_VALS = None


def _e4m3_tables():
    global _E4M3_VALS
    if _E4M3_VALS is None:
        codes = np.arange(127, dtype=np.int64)  # 0x00..0x7E (0x7F = NaN)
        e = (codes >> 3) & 0xF
        m = (codes & 0x7).astype(np.float64)
        _E4M3_VALS = np.where(e == 0, (m / 8.0) * 2.0 ** -6,
                              (1.0 + m / 8.0) * 2.0 ** (e.astype(np.float64) - 7))
    return _E4M3_VALS


def _e4m3_quantize(x):
    """round-to-nearest e4m3fn; returns uint8 codes."""
    vals = _e4m3_tables()
    mids = (vals[:-1] + vals[1:]) / 2.0
    code = np.searchsorted(mids, np.abs(x), side="right")
    np.minimum(code, 126, out=code)
    return code.astype(np.uint8) | ((x < 0).astype(np.uint8) << 7)


def _host_prep(embeddings, survival_times):
    E = np.asarray(embeddings, dtype=np.float64)
    t = np.asarray(survival_times, dtype=np.float32)
    q = np.quantile(t.astype(np.float64), [0.25, 0.5, 0.75])
    rg = (t[:, None].astype(np.float64) >= q[None, :]).sum(axis=1)
    counts = np.bincount(rg, minlength=G)
    assert (counts == N // G).all(), counts
    perm = np.argsort(rg, kind="stable")
    Z = E[perm]
    Z /= np.maximum(np.sqrt((Z * Z).sum(axis=1, keepdims=True)), 1e-12)
    codes = _e4m3_quantize(Z * ZSCALE)       # [N, D] uint8

    ZT = np.ascontiguousarray(codes.T)       # [D, N]
    ztA = np.ascontiguousarray(ZT[0:256].reshape(2, 128, N).transpose(1, 0, 2))
    ztB = np.ascontiguousarray(ZT[256:512].reshape(2, 128, N).transpose(1, 0, 2))
    idm = np.eye(128, dtype=np.float32)

    in_maps = []
    for c in range(NCORES):
        sh = c * BS
        in_maps.append({
            "zA": np.ascontiguousarray(np.roll(ztA, -sh, axis=2)),
            "zB": np.ascontiguousarray(np.roll(ztB, -sh, axis=2)),
            "idm": idm,
        })
    return in_maps


def kernel(embeddings, survival_times, censor):
    from concourse.bass_utils import run_bass_kernel_spmd

    nc = _get_built()
    in_maps = _host_prep(embeddings, survival_times)
    res = run_bass_kernel_spmd(nc, in_maps, list(range(NCORES)))

    den = np.zeros(N, dtype=np.float64)
    pos = np.zeros(N, dtype=np.float64)
    for c in range(NCORES):
        rso = np.asarray(res.results[c]["rso"], dtype=np.float64)  # [128, 80]
        cso = np.asarray(res.results[c]["cso"], dtype=np.float64).reshape(-1)
        for k, sb in enumerate(SBS):
            for h, (i0, j0) in enumerate(sb):
                gi = (i0 + c) % NB
                gj = (j0 + c) % NB
                ri = slice(gi * BS, (gi + 1) * BS)
                rj = slice(gj * BS, (gj + 1) * BS)
                csv = cso[k * 1024 + h * 512: k * 1024 + (h + 1) * 512]
                if i0 == j0:
                    rsv = csv  # diag block: row sums = col sums
                else:
                    rsv = rso[:, [k * 8 + 2 * r + h for r in range(4)]].T.reshape(BS)
                den[ri] += rsv
                same = (gi // 4) == (gj // 4)
                if same:
                    pos[ri] += rsv
                if i0 != j0:
                    den[rj] += csv
                    if same:
                        pos[rj] += csv
        for kk, i0 in ((0, 0), (1, 8)):
            gi = (i0 + c) % NB
            ri = slice(gi * BS, (gi + 1) * BS)
            dvv = rso[:, [72 + kk * 4 + r for r in range(4)]].T.reshape(BS)
            den[ri] -= dvv
            pos[ri] -= dvv

    loss = np.mean(np.log(den) - np.log(pos))
    return np.float32(loss)


# revision 9
# speedup vs baseline: 1.0250x; 1.0250x over previous
"""Distributed Trainium2 kernel for nn_ContrastiveLoss (survival contrastive loss).

Strategy (8 NeuronCores, symmetric block-pair decomposition):
  host: quantile-bin rows into 4 risk groups, stable-sort rows by group,
        L2-normalize, scale x64 and quantize to fp8-e4m3; build z^T in
        DoubleRow layout [128, 2, N]; each core gets a column-rotated copy
        (rotation by c*512) so one static SPMD program covers all 136
        unordered 512x512 block-pairs of the symmetric sim matrix
        (template pairs: i<=j, (i+j) mod 16 in {0,1} -> 17 pairs/core).
  device (per core): per block-pair chunk [128 rows x 512 cols]:
        2 fp8 DoubleRow matmuls (K=256 each) -> psum; ACT exp(10*s-10)
        over both halves of a super-block in one [128,2,512] instruction
        -> bf16 SBUF; DVE row-sum reduce (off-diag pairs); ones-matmul
        col-sums accumulated in psum (= row sums of the transposed block,
        by symmetry); fused mask+reduce diag extraction for the 2 diagonal
        template blocks; gpsimd stages colsum psum rows to SBUF.
  host: scatter row/col partial sums into den/pos by risk group (f64),
        subtract extracted diagonal, loss = mean(ln den - ln pos).
"""
import sys

sys.path.insert(0, "/opt/trn_rl_repo")
import numpy as np

N, D, G, NCORES = 8192, 512, 4, 8
NB = 16          # number of 512-row/col blocks
BS = 512         # block size
TEMP = 0.1
ZSCALE = 64.0    # fp8 pre-scale (avoids subnormals)

# super-blocks: groups of (1 or 2) template pairs sharing one psum tile.
# template = {(i,j): i<=j, (i+j) mod 16 in {0,1}}; diag pairs first.
SBS = [
    [(0, 0), (0, 1)],
    [(8, 8), (8, 9)],
    [(1, 15), (2, 15)],
    [(2, 14), (3, 14)],
    [(3, 13), (4, 13)],
    [(4, 12), (5, 12)],
    [(5, 11), (6, 11)],
    [(6, 10), (7, 10)],
    [(7, 9)],
]
# column-block DMA order: prefix-feasible for the SB sequence above
LOAD_ORDER = [0, 1, 8, 9, 2, 15, 3, 14, 4, 13, 5, 12, 6, 11, 7, 10]

_built = None


def _build():
    from concourse import bacc, tile, mybir

    nc = bacc.Bacc(None, target_bir_lowering=False)
    f32 = mybir.dt.float32
    bf16 = mybir.dt.bfloat16
    fp8 = mybir.dt.float8e4
    u8 = mybir.dt.uint8
    AF = mybir.ActivationFunctionType
    AX = mybir.AxisListType
    ADD = mybir.AluOpType.add
    MUL = mybir.AluOpType.mult
    DR = mybir.MatmulPerfMode.DoubleRow

    zA = nc.dram_tensor("zA", [128, 2, N], u8, kind="ExternalInput")
    zB = nc.dram_tensor("zB", [128, 2, N], u8, kind="ExternalInput")
    idm = nc.dram_tensor("idm", [128, 128], f32, kind="ExternalInput")
    rso = nc.dram_tensor("rso", [128, 80], f32, kind="ExternalOutput")
    cso = nc.dram_tensor("cso", [1, 8704], f32, kind="ExternalOutput")

    ACT_SCALE = (1.0 / TEMP) / (ZSCALE * ZSCALE)

    with tile.TileContext(nc) as tc:
        with tc.tile_pool(name="zt", bufs=1) as ztp, \
             tc.tile_pool(name="cst", bufs=1) as cstp, \
             tc.tile_pool(name="eb", bufs=4) as ebp, \
             tc.tile_pool(name="sc", bufs=2) as scp, \
             tc.tile_pool(name="pm", bufs=3, space="PSUM") as pmp, \
             tc.tile_pool(name="pc", bufs=1, space="PSUM") as pcp:

            # preload exp/ln act table once (set 6)
            nc.scalar.add_instruction(
                mybir.InstLoadActFuncSet(
                    name=nc.get_next_instruction_name(),
                    act_func_set_id=6, ins=[], outs=[]))

            zAs = ztp.tile([128, 2, N], fp8, name="zAs")
            zBs = ztp.tile([128, 2, N], fp8, name="zBs")
            for cb in LOAD_ORDER:
                cs = slice(cb * BS, (cb + 1) * BS)
                nc.sync.dma_start(zAs[:, :, cs], zA[:, :, cs].bitcast(fp8))
                nc.sync.dma_start(zBs[:, :, cs], zB[:, :, cs].bitcast(fp8))

            idmt = cstp.tile([128, 128], f32, name="idmt")
            nc.sync.dma_start(idmt[:], idm[:])
            onest = cstp.tile([128, 128], bf16, name="onest")
            nc.vector.memset(onest[:], 1.0)
            biasm = cstp.tile([128, 1], f32, name="biasm")
            nc.vector.memset(biasm[:], -10.0)
            rst = cstp.tile([128, 80], f32, name="rst")
            nc.vector.memset(rst[:], 0.0)
            csb = cstp.tile([1, 8704], f32, name="csb")

            zts = (zAs, zBs)

            def emit_sim(k, sb, r, pm):
                for h, (bi, bj) in enumerate(sb):
                    ls = slice(bi * BS + r * 128, bi * BS + (r + 1) * 128)
                    rs_ = slice(bj * BS, (bj + 1) * BS)
                    for t in range(2):
                        nc.tensor.matmul(pm[:, h, :], zts[t][:, :, ls],
                                         zts[t][:, :, rs_],
                                         start=(t == 0), stop=(t == 1),
                                         perf_mode=DR)

            # chunks whose row sums ride the ACT accumulator (DVE relief):
            # two single-half exp instructions instead of one double
            ACT_ACCUM_CHUNKS = {(2, 1), (4, 1), (6, 1)}

            def emit_post(k, sb, r, pm, cst):
                nh = len(sb)
                eb = ebp.tile([128, 2, 512], bf16, tag="eb", name=f"eb{k}_{r}")
                if (k, r) in ACT_ACCUM_CHUNKS and nh == 2 and k not in (0, 1):
                    for h in range(2):
                        nc.scalar.activation(
                            eb[:, h, :], pm[:, h, :], AF.Exp,
                            bias=biasm[:], scale=ACT_SCALE,
                            accum_out=rst[:, k * 8 + r * 2 + h:
                                          k * 8 + r * 2 + h + 1])
                    for h in range(nh):
                        nc.tensor.matmul(cst[:, h, :], onest[:], eb[:, h, :],
                                         start=(r == 0), stop=(r == 3))
                    return
                nc.scalar.activation(eb[:, :nh, :], pm[:, :nh, :], AF.Exp,
                                     bias=biasm[:], scale=ACT_SCALE)
                if k in (0, 1):
                    # diag pair in half 0: its row sums = col sums (symmetric
                    # block) -> host reads them from cso; reduce half 1 only.
                    nc.vector.tensor_reduce(
                        rst[:, k * 8 + r * 2 + 1: k * 8 + r * 2 + 2],
                        eb[:, 1:2, :], AX.X, ADD)
                    # diag extraction: mask-mul then row-reduce
                    sc = scp.tile([128, 128], bf16, tag="sc", name=f"sc{k}_{r}")
                    nc.vector.tensor_mul(sc[:], eb[:, 0, r * 128:(r + 1) * 128],
                                         idmt[:])
                    nc.vector.tensor_reduce(
                        rst[:, 72 + k * 4 + r: 73 + k * 4 + r],
                        sc[:], AX.X, ADD)
                else:
                    nc.vector.tensor_reduce(
                        rst[:, k * 8 + r * 2: k * 8 + r * 2 + nh],
                        eb[:, :nh, :], AX.X, ADD)
                for h in range(nh):
                    nc.tensor.matmul(cst[:, h, :], onest[:], eb[:, h, :],
                                     start=(r == 0), stop=(r == 3))
                if r == 3:
                    # stage colsum psum row to SBUF (gpsimd cannot read psum)
                    nc.vector.tensor_copy(csb[0:1, k * 1024: k * 1024 + nh * 512],
                                          cst[0:1, :nh, :])

            # software-pipelined emission: sim(r+1) goes ahead of post(r) so
            # the PE never sits behind an ACT it doesn't depend on
            pend = None
            for k, sb in enumerate(SBS):
                cst = pcp.tile([128, 2, 512], f32, tag="cst", name=f"cst{k}")
                for r in range(4):
                    pm = pmp.tile([128, 2, 512], f32, tag="pm",
                                  name=f"pm{k}_{r}")
                    emit_sim(k, sb, r, pm)
                    if pend is not None:
                        emit_post(*pend)
                    pend = (k, sb, r, pm, cst)
            emit_post(*pend)
            nc.sync.dma_start(cso[:], csb[:])
            nc.sync.dma_start(rso[:], rst[:])

    nc.finalize()
    return nc


def _get_built():
    global _built
    if _built is None:
        _built = _build()
    return _built


_E4M3_VALS = None


def _e4m3_tables():
    global _E4M3_VALS
    if _E4M3_VALS is None:
        codes = np.arange(127, dtype=np.int64)  # 0x00..0x7E (0x7F = NaN)
        e = (codes >> 3) & 0xF
        m = (codes & 0x7).astype(np.float64)
        _E4M3_VALS = np.where(e == 0, (m / 8.0) * 2.0 ** -6,
                              (1.0 + m / 8.0) * 2.0 ** (e.astype(np.float64) - 7))
    return _E4M3_VALS


def _e4m3_quantize(x):
    """round-to-nearest e4m3fn; returns uint8 codes."""
    vals = _e4m3_tables()
    mids = (vals[:-1] + vals[1:]) / 2.0
    code = np.searchsorted(mids, np.abs(x), side="right")
    np.minimum(code, 126, out=code)
    return code.astype(np.uint8) | ((x < 0).astype(np.uint8) << 7)


def _host_prep(embeddings, survival_times):
    E = np.asarray(embeddings, dtype=np.float64)
    t = np.asarray(survival_times, dtype=np.float32)
    q = np.quantile(t.astype(np.float64), [0.25, 0.5, 0.75])
    rg = (t[:, None].astype(np.float64) >= q[None, :]).sum(axis=1)
    counts = np.bincount(rg, minlength=G)
    assert (counts == N // G).all(), counts
    perm = np.argsort(rg, kind="stable")
    Z = E[perm]
    Z /= np.maximum(np.sqrt((Z * Z).sum(axis=1, keepdims=True)), 1e-12)
    codes = _e4m3_quantize(Z * ZSCALE)       # [N, D] uint8

    ZT = np.ascontiguousarray(codes.T)       # [D, N]
    ztA = np.ascontiguousarray(ZT[0:256].reshape(2, 128, N).transpose(1, 0, 2))
    ztB = np.ascontiguousarray(ZT[256:512].reshape(2, 128, N).transpose(1, 0, 2))
    idm = np.eye(128, dtype=np.float32)

    in_maps = []
    for c in range(NCORES):
        sh = c * BS
        in_maps.append({
            "zA": np.ascontiguousarray(np.roll(ztA, -sh, axis=2)),
            "zB": np.ascontiguousarray(np.roll(ztB, -sh, axis=2)),
            "idm": idm,
        })
    return in_maps


def kernel(embeddings, survival_times, censor):
    from concourse.bass_utils import run_bass_kernel_spmd

    nc = _get_built()
    in_maps = _host_prep(embeddings, survival_times)
    res = run_bass_kernel_spmd(nc, in_maps, list(range(NCORES)))

    den = np.zeros(N, dtype=np.float64)
    pos = np.zeros(N, dtype=np.float64)
    for c in range(NCORES):
        rso = np.asarray(res.results[c]["rso"], dtype=np.float64)  # [128, 80]
        cso = np.asarray(res.results[c]["cso"], dtype=np.float64).reshape(-1)
        for k, sb in enumerate(SBS):
            for h, (i0, j0) in enumerate(sb):
                gi = (i0 + c) % NB
                gj = (j0 + c) % NB
                ri = slice(gi * BS, (gi + 1) * BS)
                rj = slice(gj * BS, (gj + 1) * BS)
                csv = cso[k * 1024 + h * 512: k * 1024 + (h + 1) * 512]
                if i0 == j0:
                    rsv = csv  # diag block: row sums = col sums
                else:
                    rsv = rso[:, [k * 8 + 2 * r + h for r in range(4)]].T.reshape(BS)
                den[ri] += rsv
                same = (gi // 4) == (gj // 4)
                if same:
                    pos[ri] += rsv
                if i0 != j0:
                    den[rj] += csv
                    if same:
                        pos[rj] += csv
        for kk, i0 in ((0, 0), (1, 8)):
            gi = (i0 + c) % NB
            ri = slice(gi * BS, (gi + 1) * BS)
            dvv = rso[:, [72 + kk * 4 + r for r in range(4)]].T.reshape(BS)
            den[ri] -= dvv
            pos[ri] -= dvv

    loss = np.mean(np.log(den) - np.log(pos))
    return np.float32(loss)
